# revision 1
# baseline (speedup 1.0000x reference)
"""GCN (gather/segment-sum message passing) + mean-pool + MLP on 8 TRN2 cores.

Strategy (data-parallel over graphs, per the sharding hint):
 - nodes/graphs are sharded contiguously across 8 cores (batch is sorted);
   every edge is owned by the core owning its TARGET (col) node.
 - launch 1: each core computes y = rsqrt(deg) * (x @ W_gcn) for its node
   shard (host stages x pre-transposed so the PE contracts over in_dim).
 - host assembles the full y table (node-id order) + per-bank zero rows.
 - launch 2: per core, per source-bank (int16 gather indices limit a table
   to 32k rows -> 4 banks), edges are organized into "prefix rounds": nodes
   sorted by per-bank in-degree, round r gathers the r-th in-edge source row
   of every node that has one. Each round's dma_gather output tile is then
   POSITION-ALIGNED with the accumulator (node rank i -> partition i%128,
   column i//128), so aggregation is plain DVE adds - no scatter at all.
   Bank partials are merged by a small permute-gather through HBM scratch.
   Then z = relu(dinv*acc + b), graph mean-pool via one-hot PSUM matmuls,
   and the 64->64->2 MLP + sigmoid, all on-chip. Output (64,2) per core.
"""

import os
import sys

sys.path.insert(0, "/opt/trn_rl_repo")

import numpy as np

import concourse.bacc as bacc
import concourse.bass as bass
import concourse.mybir as mybir
import concourse.tile as tile
from concourse.bass_utils import run_bass_kernel_spmd
from concourse.vector_clock import ScopedClock

NC = 8          # cores
NB = 4          # source banks (int16 gather index limit)
CH = 1024       # gather chunk (slots per dma_gather; SWDGE ring caps ~128 descs/engine)
NQ = 2          # SWDGE queues for dma_gather round-robin
SUP = 512       # idx super-tile columns (x16 idxs)
P = 128
HID = 64
F32 = mybir.dt.float32
I16 = mybir.dt.int16

LAST_RUN_INFO = {}


def _split_multiwaits(nc, max_waits=1):
    """This walrus build rejects >1 semaphore wait per instruction; hoist
    extra waits onto same-engine NOPs placed immediately before."""
    import concourse.mybir as mb
    for f in nc.m.functions:
        for blk in f.blocks:
            insts = blk.instructions
            newlist = []
            changed = False
            for inst in insts:
                si = inst.sync_info
                waits = list(si.on_wait) if si is not None and si.on_wait else []
                if len(waits) > max_waits:
                    si.on_wait = waits[-max_waits:]
                    extra = waits[:-max_waits]
                    while extra:
                        nop = mb.InstNoOp(
                            name=f"I-mwsplit-{nc.next_id()}",
                            sync_info=mb.SyncInfo(on_wait=extra[:max_waits], on_update=[]),
                            engine=inst.engine,
                            bass_nofuse=True,
                        )
                        newlist.append(nop)
                        extra = extra[max_waits:]
                    changed = True
                newlist.append(inst)
            if changed:
                insts.clear()
                insts.extend(newlist)


_COMPILED = set()


def _run(nc, in_maps, trace=False):
    if id(nc) not in _COMPILED:
        nc.compile()
        _split_multiwaits(nc)
        _COMPILED.add(id(nc))
    kw = {}
    if trace:
        kw = dict(trace=True)
    try:
        return run_bass_kernel_spmd(nc, in_maps, list(range(NC)), **kw)
    except Exception:
        # transient device-unrecoverable (wedged core from an earlier run)
        import time as _time
        _time.sleep(10)
        return run_bass_kernel_spmd(nc, in_maps, list(range(NC)), **kw)


def _pjrt_runner(nc, in_maps):
    """Build the shard_map-jitted bass_exec callable ONCE with device-resident
    inputs; returns run_once() whose wall time is dispatch + device exec only
    (fresh donated zero-outputs are re-supplied per call; for benchmarking)."""
    import jax
    import numpy as _np
    from concourse import bass2jax as b2j

    b2j.install_neuronx_cc_hook()
    partition_name = nc.partition_id_tensor.name if nc.partition_id_tensor else None
    in_names, out_names, out_avals, zero_outs = [], [], [], []
    for alloc in nc.m.functions[0].allocations:
        if not isinstance(alloc, mybir.MemoryLocationSet):
            continue
        name = alloc.memorylocations[0].name
        if alloc.kind == "ExternalInput":
            if name != partition_name:
                in_names.append(name)
        elif alloc.kind == "ExternalOutput":
            shape = tuple(alloc.tensor_shape)
            dtype = mybir.dt.np(alloc.dtype)
            out_names.append(name)
            out_avals.append(jax.core.ShapedArray(shape, dtype))
            zero_outs.append(_np.zeros(shape, dtype))
    n_params, n_outs = len(in_names), len(out_avals)
    all_in = list(in_names) + out_names + ([partition_name] if partition_name else [])

    def _body(*args):
        operands = list(args)
        if partition_name is not None:
            operands.append(b2j.partition_id_tensor())
        outs = b2j._bass_exec_p.bind(
            *operands, out_avals=tuple(out_avals), in_names=tuple(all_in),
            out_names=tuple(out_names), lowering_input_output_aliases=(),
            sim_require_finite=True, sim_require_nnan=True, nc=nc)
        return tuple(outs)

    devices = jax.devices()[:NC]
    mesh = b2j.Mesh(_np.asarray(devices), ("core",))
    donate = tuple(range(n_params, n_params + n_outs))
    sharded = jax.jit(
        b2j.shard_map(_body, mesh=mesh,
                      in_specs=(b2j.PartitionSpec("core"),) * (n_params + n_outs),
                      out_specs=(b2j.PartitionSpec("core"),) * n_outs,
                      check_rep=False),
        donate_argnums=donate, keep_unused=True)
    concat_in = [
        jax.device_put(
            _np.concatenate([_np.asarray(m[name]) for m in in_maps], axis=0))
        for name in in_names
    ]
    for a in concat_in:
        a.block_until_ready()

    def run_once():
        zs = [_np.zeros((NC * z.shape[0], *z.shape[1:]), z.dtype) for z in zero_outs]
        outs = sharded(*concat_in, *zs)
        for o in outs:
            o.block_until_ready()
        return outs

    return run_once


# ---------------------------------------------------------------- launch 1


def _build_launch1(C):
    """y_tile = dinv * (x @ W);  x staged transposed [128(in), C*128(node)]."""
    nc = bacc.Bacc("TRN2", target_bir_lowering=False, debug=False)
    xT = nc.declare_dram_parameter("xT", [P, C * P], F32, isOutput=False)
    degn = nc.declare_dram_parameter("degn", [P, C], F32, isOutput=False)
    w = nc.declare_dram_parameter("w", [P, HID], F32, isOutput=False)
    ysb = nc.declare_dram_parameter("ysb", [P, C * HID], F32, isOutput=True)

    reps = int(os.environ.get("GCN_REPS", "1"))
    with tile.TileContext(nc) as tc:
        with (
            tc.tile_pool(name="sb", bufs=1) as sb,
            tc.tile_pool(name="sbx", bufs=3) as sbx,
            tc.tile_pool(name="ps", bufs=4, space="PSUM") as psp,
        ):
          for _rep in range(reps):
            w_t = sb.tile([P, HID], F32)
            nc.scalar.dma_start(out=w_t[:], in_=w[:, :])
            deg_t = sb.tile([P, C], F32)
            nc.scalar.dma_start(out=deg_t[:], in_=degn[:, :])
            dinv = sb.tile([P, C], F32)
            nc.scalar.activation(dinv[:], deg_t[:], mybir.ActivationFunctionType.Sqrt)
            nc.vector.reciprocal(dinv[:], dinv[:])
            y_t = sb.tile([P, C, HID], F32)
            for t in range(C):
                xt_t = sbx.tile([P, P], F32)
                nc.scalar.dma_start(out=xt_t[:], in_=xT[:, t * P:(t + 1) * P])
                ps = psp.tile([P, HID], F32, space="PSUM")
                nc.tensor.matmul(out=ps[:], lhsT=xt_t[:], rhs=w_t[:],
                                 start=True, stop=True)
                nc.vector.tensor_tensor(
                    out=y_t[:, t, :], in0=ps[:],
                    in1=dinv[:, t:t + 1].broadcast_to([P, HID]),
                    op=mybir.AluOpType.mult)
            nc.scalar.dma_start(out=ysb[:, :], in_=y_t[:].rearrange("p c h -> p (c h)"))
    return nc


# ---------------------------------------------------------------- launch 2


def _build_launch2(C, VB, bank_chunks, merge_chunks, n_w16):
    """bank_chunks: per bank, list of (idx_off16, nidx, [(gcol, zcol, ncols)..])
    merge_chunks: per bank, list of (idx_off16, nidx, gcol0, zcol0)
    n_w16: total idx columns (int16 words / 16)."""
    nc = bacc.Bacc("TRN2", target_bir_lowering=False, debug=False,
                   num_swdge_queues=NQ)
    ytab = nc.declare_dram_parameter("ytab", [NB * VB, HID], F32, isOutput=False)
    idxs = nc.declare_dram_parameter("idxs", [P, n_w16], I16, isOutput=False)
    degz = nc.declare_dram_parameter("degz", [P, C], F32, isOutput=False)
    gl = nc.declare_dram_parameter("gl", [P, C], F32, isOutput=False)
    iota = nc.declare_dram_parameter("iota", [P, HID], F32, isOutput=False)
    brep = nc.declare_dram_parameter("brep", [P, HID], F32, isOutput=False)
    w1a = nc.declare_dram_parameter("w1a", [P, HID], F32, isOutput=False)
    w2a = nc.declare_dram_parameter("w2a", [P, 2], F32, isOutput=False)
    iden = nc.declare_dram_parameter("iden", [P, P], F32, isOutput=False)
    out = nc.declare_dram_parameter("out", [HID, 2], F32, isOutput=True)
    dbg = os.environ.get("GCN_DEBUG") == "1"
    if dbg:
        zdbg = nc.declare_dram_parameter("zdbg", [P, C * HID], F32, isOutput=True)
    zscr = nc.dram_tensor("zscr", [NB * P * C, HID], F32)

    reps = int(os.environ.get("GCN_REPS", "1"))
    with tile.TileContext(nc) as tc:
        with (
            tc.tile_pool(name="sb", bufs=1) as sb,
            tc.tile_pool(name="stage", bufs=int(os.environ.get("GCN_SBUFS", "3"))) as stage,
            tc.tile_pool(name="idxp", bufs=3) as idxp,
            tc.tile_pool(name="ohp", bufs=3) as ohp,
            tc.tile_pool(name="ps", bufs=1, space="PSUM") as psp,
            tc.tile_pool(name="ps2", bufs=1, space="PSUM") as psp2,
        ):
            acc = sb.tile([P, C, HID], F32, tag="acc")
            z = sb.tile([P, C, HID], F32, tag="z")
            sup_state = {"s0": -1, "tile": None}

            def get_idx(off16, w):
                if sup_state["s0"] < 0 or off16 + w > sup_state["s0"] + SUP:
                    w2 = min(SUP, n_w16 - off16)
                    t = idxp.tile([P, SUP], I16, tag="idx")
                    nc.scalar.dma_start(out=t[:, :w2], in_=idxs[:, off16:off16 + w2])
                    sup_state["s0"] = off16
                    sup_state["tile"] = t
                o = off16 - sup_state["s0"]
                return sup_state["tile"][:, o:o + w]

            gq = [0]

            def gather(dst_ap, src_ap, off16, nidx):
                it = get_idx(off16, nidx // 16)
                gi = nc.gpsimd.dma_gather(dst_ap, src_ap, it, nidx, nidx, HID,
                                          queue_num=gq[0] % NQ)
                gq[0] += 1
                return gi

            def body():
              # (indented 2: repeated GCN_REPS times for benchmarking)
              sup_state["s0"] = -1
              dump_insts = []
              for b in range(NB):
                  nc.gpsimd.memset(acc[:], 0.0)
                  for (off16, nidx, pieces) in bank_chunks[b]:
                      st = stage.tile([P, CH // P, HID], F32, tag="st")
                      gather(st[:, : nidx // P, :], ytab[b * VB:(b + 1) * VB, :],
                             off16, nidx)
                      for (gcol, zcol, ncols) in pieces:
                          nc.vector.tensor_tensor(
                              out=acc[:, zcol:zcol + ncols, :],
                              in0=acc[:, zcol:zcol + ncols, :],
                              in1=st[:, gcol:gcol + ncols, :],
                              op=mybir.AluOpType.add)
                  di = nc.scalar.dma_start(
                      out=zscr[b * P * C:(b + 1) * P * C, :],
                      in_=acc[:].rearrange("p c h -> p (c h)"))
                  dump_insts.append(di)
              # merge partials (node order): z = sum_b permute(acc_b)
              nc.gpsimd.memset(z[:], 0.0)
              for b in range(NB):
                  for (off16, nidx, gcol0, zcol0) in merge_chunks[b]:
                      st = stage.tile([P, CH // P, HID], F32, tag="st")
                      gi = gather(st[:, : nidx // P, :],
                                  zscr[b * P * C:(b + 1) * P * C, :], off16, nidx)
                      tile.add_dep_helper(gi.ins, dump_insts[b].ins, sync=True,
                                          reason="merge gather reads zscr dump")
                      nc.vector.tensor_tensor(
                          out=z[:, zcol0:zcol0 + nidx // P, :],
                          in0=z[:, zcol0:zcol0 + nidx // P, :],
                          in1=st[:, : nidx // P, :],
                          op=mybir.AluOpType.add)
              # dinv
              deg_t = sb.tile([P, C], F32)
              nc.scalar.dma_start(out=deg_t[:], in_=degz[:, :])
              dinv = sb.tile([P, C], F32)
              nc.scalar.activation(dinv[:], deg_t[:], mybir.ActivationFunctionType.Sqrt)
              nc.vector.reciprocal(dinv[:], dinv[:])
              brep_t = sb.tile([P, HID], F32)
              nc.scalar.dma_start(out=brep_t[:], in_=brep[:, :])
              for c in range(C):
                  nc.vector.tensor_tensor(
                      out=z[:, c, :], in0=z[:, c, :],
                      in1=dinv[:, c:c + 1].broadcast_to([P, HID]),
                      op=mybir.AluOpType.mult)
                  nc.vector.tensor_tensor(
                      out=z[:, c, :], in0=z[:, c, :], in1=brep_t[:],
                      op=mybir.AluOpType.add)
              zf = z[:].rearrange("p c h -> p (c h)")
              nc.scalar.activation(zf, zf, mybir.ActivationFunctionType.Relu)
              if dbg:
                  nc.scalar.dma_start(out=zdbg[:, :], in_=zf)
              # pooling: one-hot PSUM matmuls
              gl_t = sb.tile([P, C], F32)
              nc.scalar.dma_start(out=gl_t[:], in_=gl[:, :])
              iota_t = sb.tile([P, HID], F32)
              nc.scalar.dma_start(out=iota_t[:], in_=iota[:, :])
              ones_t = sb.tile([P, 1], F32)
              nc.gpsimd.memset(ones_t[:], 1.0)
              ps_sum = psp.tile([HID, HID], F32, space="PSUM", tag="pssum")
              ps_cnt = psp.tile([HID, 1], F32, space="PSUM", tag="pscnt")
              for c in range(C):
                  oh = ohp.tile([P, HID], F32, tag="oh")
                  nc.vector.tensor_tensor(
                      out=oh[:], in0=gl_t[:, c:c + 1].broadcast_to([P, HID]),
                      in1=iota_t[:], op=mybir.AluOpType.is_equal)
                  nc.tensor.matmul(out=ps_sum[:], lhsT=oh[:], rhs=z[:, c, :],
                                   start=(c == 0), stop=(c == C - 1),
                                   skip_group_check=True)
                  nc.tensor.matmul(out=ps_cnt[:], lhsT=oh[:], rhs=ones_t[:],
                                   start=(c == 0), stop=(c == C - 1),
                                   skip_group_check=True)
              cnt = sb.tile([HID, 1], F32)
              nc.vector.tensor_scalar_max(cnt[:], ps_cnt[:], 1.0)
              nc.vector.reciprocal(cnt[:], cnt[:])
              g_sb = sb.tile([HID, HID], F32)
              nc.vector.tensor_tensor(out=g_sb[:], in0=ps_sum[:],
                                      in1=cnt[:].broadcast_to([HID, HID]),
                                      op=mybir.AluOpType.mult)
              # MLP with homogeneous-coordinate bias
              iden_t = sb.tile([P, P], F32)
              nc.scalar.dma_start(out=iden_t[:], in_=iden[:, :])
              w1_t = sb.tile([P, HID], F32)
              nc.scalar.dma_start(out=w1_t[:], in_=w1a[:, :])
              w2_t = sb.tile([P, 2], F32)
              nc.scalar.dma_start(out=w2_t[:], in_=w2a[:, :])
              gT = psp2.tile([HID, HID], F32, space="PSUM", tag="tr")
              nc.tensor.transpose(out=gT[:], in_=g_sb[:], identity=iden_t[:HID, :HID])
              a1 = sb.tile([P, HID], F32)
              nc.gpsimd.memset(a1[HID:HID + 1, :], 1.0)
              nc.vector.tensor_copy(a1[:HID, :], gT[:])
              h_ps = psp2.tile([HID, HID], F32, space="PSUM", tag="mm")
              nc.tensor.matmul(out=h_ps[:], lhsT=a1[0:HID + 1, :], rhs=w1_t[0:HID + 1, :],
                               start=True, stop=True)
              h_sb = sb.tile([HID, HID], F32)
              nc.scalar.activation(h_sb[:], h_ps[:], mybir.ActivationFunctionType.Relu)
              hT = psp2.tile([HID, HID], F32, space="PSUM", tag="tr2")
              nc.tensor.transpose(out=hT[:], in_=h_sb[:], identity=iden_t[:HID, :HID])
              a2 = sb.tile([P, HID], F32)
              nc.gpsimd.memset(a2[HID:HID + 1, :], 1.0)
              nc.vector.tensor_copy(a2[:HID, :], hT[:])
              o_ps = psp2.tile([HID, 2], F32, space="PSUM", tag="mm2")
              nc.tensor.matmul(out=o_ps[:], lhsT=a2[0:HID + 1, :], rhs=w2_t[0:HID + 1, :],
                               start=True, stop=True)
              o_sb = sb.tile([HID, 2], F32)
              nc.scalar.activation(o_sb[:], o_ps[:], mybir.ActivationFunctionType.Sigmoid)
              nc.scalar.dma_start(out=out[:, :], in_=o_sb[:])

            for _rep in range(reps):
                body()
    return nc


# ---------------------------------------------------------------- host glue


def _wrap16(vals):
    """int16 stream -> [128, ceil(n/16)] ucode layout (16-wrapped, 8x repl)."""
    n = len(vals)
    w = (n + 15) // 16
    a = np.full(w * 16, -1, np.int16)
    a[:n] = vals
    blk = a.reshape(w, 16).T
    return np.tile(blk, (8, 1))


def kernel(x, edge_index, batch, W_gcn, b_gcn, W1, b1, W2, b2):
    x = np.ascontiguousarray(np.asarray(x, dtype=np.float32))
    ei = np.asarray(edge_index).astype(np.int64)
    batch_np = np.asarray(batch).astype(np.int64)
    W_gcn = np.asarray(W_gcn, np.float32); b_gcn = np.asarray(b_gcn, np.float32)
    W1 = np.asarray(W1, np.float32); b1 = np.asarray(b1, np.float32)
    W2 = np.asarray(W2, np.float32); b2 = np.asarray(b2, np.float32)

    N = x.shape[0]
    G = 512
    BS = (N + NB - 1) // NB          # nodes per source bank
    VB = BS + 1                      # +1 zero row per bank
    row = ei[0].astype(np.int64)
    col = ei[1].astype(np.int64)
    # self loops appended
    sl = np.arange(N, dtype=np.int64)
    row2 = np.concatenate([row, sl])
    col2 = np.concatenate([col, sl])
    deg = np.bincount(col2, minlength=N).astype(np.float32)  # >=1 always

    gpc = G // NC
    gb = np.searchsorted(batch_np, np.arange(0, G + 1, gpc))
    Ncs = np.diff(gb)
    C = int((Ncs.max() + P - 1) // P)

    # ---------------- launch 1: y shards
    in1 = []
    for c in range(NC):
        lo, hi = int(gb[c]), int(gb[c + 1])
        n = hi - lo
        xT = np.zeros((P, C * P), np.float32)
        xT[:, :n] = x[lo:hi].T
        dg = np.ones((P, C), np.float32)
        dgf = dg.reshape(-1, order="F")      # (p,t) -> t*128+p
        dgf[:n] = deg[lo:hi]
        dg = dgf.reshape(C, P).T.copy()
        in1.append({"xT": xT, "degn": dg, "w": W_gcn})
    nc1 = _build_launch1(C)
    trace = os.environ.get("GCN_TRACE") == "1"
    r1 = _run(nc1, in1, trace=trace)
    LAST_RUN_INFO["exec1_ns"] = r1.exec_time_ns
    y_full = np.empty((N, HID), np.float32)
    for c in range(NC):
        lo, hi = int(gb[c]), int(gb[c + 1])
        ys = r1.results[c]["ysb"].reshape(P, C, HID).transpose(1, 0, 2).reshape(-1, HID)
        y_full[lo:hi] = ys[: hi - lo]
    ytab = np.zeros((NB * VB, HID), np.float32)
    for b in range(NB):
        nlo, nhi = b * BS, min((b + 1) * BS, N)
        ytab[b * VB: b * VB + (nhi - nlo)] = y_full[nlo:nhi]

    # ---------------- per-core schedules (common across cores)
    core_data = []
    for c in range(NC):
        lo, hi = int(gb[c]), int(gb[c + 1])
        m = (col2 >= lo) & (col2 < hi)
        r_c = row2[m]
        cl = (col2[m] - lo).astype(np.int64)
        bank = np.minimum(r_c // BS, NB - 1)
        core_data.append((lo, hi, r_c, cl, bank))

    # common round schedule per bank: N_br = max over cores of roundup128(n_br)
    nbr_all = []          # [NB][core] -> array of n_br
    for b in range(NB):
        per_core = []
        for c in range(NC):
            lo, hi, r_c, cl, bank = core_data[c]
            nloc = hi - lo
            degb = np.bincount(cl[bank == b], minlength=nloc)
            if degb.max() == 0:
                per_core.append(np.zeros(0, np.int64))
                continue
            h = np.bincount(degb)            # h[d] = #nodes with degb == d
            nbr = nloc - np.cumsum(h)[:-1] if len(h) > 1 else np.zeros(0, np.int64)
            # n_br = #{deg_b > r} for r = 0..max-1
            nbr = (nloc - np.cumsum(h))[:len(h) - 1]
            per_core.append(np.asarray(nbr, np.int64))
        nbr_all.append(per_core)
    bank_rounds = []      # [NB] -> padded common N_br (cols of 128)
    for b in range(NB):
        R = max((len(a) for a in nbr_all[b]), default=0)
        Nbr = np.zeros(R, np.int64)
        for a in nbr_all[b]:
            aa = np.zeros(R, np.int64)
            aa[:len(a)] = a
            Nbr = np.maximum(Nbr, ((aa + P - 1) // P) * P)
        bank_rounds.append(Nbr)

    # chunk schedule (common): per bank, chunks of <=CH slots + round pieces
    bank_chunks = []
    bank_off16 = []       # idx tensor column offset for each bank stream
    off16 = 0
    for b in range(NB):
        Nbr = bank_rounds[b]
        S = int(Nbr.sum())
        starts = np.concatenate([[0], np.cumsum(Nbr)])
        chunks = []
        pos = 0
        while pos < S:
            ln = min(CH, S - pos)
            pieces = []
            for r in range(len(Nbr)):
                a = max(pos, starts[r]); e = min(pos + ln, starts[r + 1])
                if a < e:
                    pieces.append((int((a - pos) // P), int((a - starts[r]) // P),
                                   int((e - a) // P)))
            chunks.append((off16 + pos // 16, int(ln), pieces))
            pos += ln
        bank_chunks.append(chunks)
        bank_off16.append(off16)
        off16 += S // 16
    # merge chunks (common): C*128 idxs per bank
    merge_chunks = []
    merge_off16 = []
    for b in range(NB):
        Sm = C * P
        chunks = []
        pos = 0
        while pos < Sm:
            ln = min(CH, Sm - pos)
            chunks.append((off16 + pos // 16, int(ln), int(pos // P), int(pos // P)))
            pos += ln
        merge_chunks.append(chunks)
        merge_off16.append(off16)
        off16 += Sm // 16
    n_w16 = off16

    # ---------------- per-core idx streams
    in2 = []
    iota64 = np.tile(np.arange(HID, dtype=np.float32), (P, 1))
    brep = np.tile(b_gcn[None, :], (P, 1)).astype(np.float32)
    w1a = np.zeros((P, HID), np.float32); w1a[:HID] = W1; w1a[HID] = b1
    w2a = np.zeros((P, 2), np.float32); w2a[:HID] = W2; w2a[HID] = b2
    iden = np.eye(P, dtype=np.float32)
    for c in range(NC):
        lo, hi, r_c, cl, bank = core_data[c]
        nloc = hi - lo
        idxbuf = np.empty(n_w16 * 16, np.int16)
        for b in range(NB):
            Nbr = bank_rounds[b]
            S = int(Nbr.sum())
            starts = np.concatenate([[0], np.cumsum(Nbr)])
            stream = np.full(S, BS, np.int16)          # dummy -> zero row
            mb = bank == b
            rb, clb = r_c[mb], cl[mb]
            degb = np.bincount(clb, minlength=nloc)
            order = np.argsort(-degb, kind="stable")   # bank-rank -> node
            rank = np.empty(nloc, np.int64)
            rank[order] = np.arange(nloc)
            rk = rank[clb]
            o = np.lexsort((np.arange(len(rk)), rk))
            rk_s, src_s = rk[o], (rb[o] - b * BS)
            grp_start = np.searchsorted(rk_s, rk_s)    # first occurrence index
            j = np.arange(len(rk_s)) - grp_start
            stream[starts[j] + rk_s] = src_s.astype(np.int16)
            idxbuf[bank_off16[b] * 16: bank_off16[b] * 16 + S] = stream
            # merge idx for this bank: node order -> acc_b row
            jb = rank                                   # node i -> bank rank
            mrow = (jb % P) * C + (jb // P)
            mstream = np.zeros(C * P, np.int16)
            mstream[:nloc] = mrow.astype(np.int16)
            idxbuf[merge_off16[b] * 16: merge_off16[b] * 16 + C * P] = mstream
        idxw = _wrap16(idxbuf)                          # [32, n_w16]
        dgz = np.ones(C * P, np.float32); dgz[:nloc] = deg[lo:hi]
        glv = np.full(C * P, float(HID), np.float32)
        glv[:nloc] = (batch_np[lo:hi] - c * gpc).astype(np.float32)
        in2.append({
            "ytab": ytab, "idxs": idxw,
            "degz": dgz.reshape(C, P).T.copy(),
            "gl": glv.reshape(C, P).T.copy(),
            "iota": iota64, "brep": brep, "w1a": w1a, "w2a": w2a, "iden": iden,
        })

    LAST_RUN_INFO["launch2_args"] = (C, VB, bank_chunks, merge_chunks, n_w16)
    LAST_RUN_INFO["in2"] = in2
    LAST_RUN_INFO["in1"] = in1
    LAST_RUN_INFO["C"] = C
    nc2 = _build_launch2(C, VB, bank_chunks, merge_chunks, n_w16)
    r2 = _run(nc2, in2, trace=trace)
    LAST_RUN_INFO["exec2_ns"] = r2.exec_time_ns
    if os.environ.get("GCN_DEBUG") == "1":
        LAST_RUN_INFO["zdbg"] = [r2.results[c]["zdbg"].reshape(P, C, HID) for c in range(NC)]
        LAST_RUN_INFO["gb"] = gb
        LAST_RUN_INFO["C"] = C
    out = np.concatenate([r2.results[c]["out"] for c in range(NC)], axis=0)
    return out[:G].astype(np.float32)



# revision 7
# speedup vs baseline: 1.4003x; 1.4003x over previous
"""GCN (gather/segment-sum message passing) + mean-pool + MLP on 8 TRN2 cores.

v2a: single jitted device chain per kernel() call:
  - host: deg/dinv, x_scaled = dinv*x staged per-core in padded blocks (bf16),
    edge schedules (prefix-round gather streams, int16 idx tables).
  - device (one jax.jit over an 8-core mesh):
      y = x_shard @ W           (XLA matmul, f32 accum)   [BLK, 64] per core
      ytab = all_gather(y)      (tiled -> [8*BLK, 64] replicated)
      out = bass_exec(launch2)  (aggregation via SWDGE dma_gather prefix
                                 rounds, position-aligned DVE adds, HBM-scratch
                                 bank merge, one-hot-matmul graph pooling, MLP)
  - banks = core PAIRS (2*BLK = 25600 rows < int16 range), dummy zero rows
    live in each core's padding block.
Eliminates: separate launch1 NEFF, host roundtrip between launches, and the
205MB replicated-ytab host->device re-ship of v1.
"""

import os
import sys

sys.path.insert(0, "/opt/trn_rl_repo")

import numpy as np
import ml_dtypes

import concourse.bacc as bacc
import concourse.mybir as mybir
import concourse.tile as tile

NC = 8          # cores
NB = 4          # source banks (int16 gather index limit; bank = core pair)
CH = 1024       # gather chunk (slots per dma_gather; SWDGE ring caps ~64 descs/engine)
NQ = 2          # SWDGE queues for dma_gather round-robin
SUP = 512       # idx super-tile columns (x16 idxs)
P = 128
HID = 64
G = 512
F32 = mybir.dt.float32
I16 = mybir.dt.int16
BF16 = ml_dtypes.bfloat16

LAST_RUN_INFO = {}


def _split_multiwaits(nc, max_waits=1):
    """This walrus build rejects >1 semaphore wait per instruction; hoist
    extra waits onto same-engine NOPs placed immediately before."""
    import concourse.mybir as mb
    for f in nc.m.functions:
        for blk in f.blocks:
            insts = blk.instructions
            newlist = []
            changed = False
            for inst in insts:
                si = inst.sync_info
                waits = list(si.on_wait) if si is not None and si.on_wait else []
                if len(waits) > max_waits:
                    si.on_wait = waits[-max_waits:]
                    extra = waits[:-max_waits]
                    while extra:
                        nop = mb.InstNoOp(
                            name=f"I-mwsplit-{nc.next_id()}",
                            sync_info=mb.SyncInfo(on_wait=extra[:max_waits], on_update=[]),
                            engine=inst.engine,
                            bass_nofuse=True,
                        )
                        newlist.append(nop)
                        extra = extra[max_waits:]
                    changed = True
                newlist.append(inst)
            if changed:
                insts.clear()
                insts.extend(newlist)


# ---------------------------------------------------------------- launch2


def _build_launch2(C, BLK, bank_chunks, merge_chunks, n_w16):
    """bank_chunks: per bank, list of (idx_off16, nidx, [(gcol, zcol, ncols)..])
    merge_chunks: per bank, list of (idx_off16, nidx, gcol0, zcol0)
    n_w16: total idx columns (int16 words / 16)."""
    nc = bacc.Bacc("TRN2", target_bir_lowering=False, debug=False,
                   num_swdge_queues=NQ)
    VB = 2 * BLK
    ytab = nc.declare_dram_parameter("ytab", [NC * BLK, HID], F32, isOutput=False)
    idxs = nc.declare_dram_parameter("idxs", [P, n_w16], I16, isOutput=False)
    degz = nc.declare_dram_parameter("degz", [P, C], F32, isOutput=False)
    gl = nc.declare_dram_parameter("gl", [P, C], F32, isOutput=False)
    iota = nc.declare_dram_parameter("iota", [P, HID], F32, isOutput=False)
    brep = nc.declare_dram_parameter("brep", [P, HID], F32, isOutput=False)
    w1a = nc.declare_dram_parameter("w1a", [P, HID], F32, isOutput=False)
    w2a = nc.declare_dram_parameter("w2a", [P, 2], F32, isOutput=False)
    iden = nc.declare_dram_parameter("iden", [P, P], F32, isOutput=False)
    out = nc.declare_dram_parameter("out", [HID, 2], F32, isOutput=True)
    dbg = os.environ.get("GCN_DEBUG") == "1"
    if dbg:
        zdbg = nc.declare_dram_parameter("zdbg", [P, C * HID], F32, isOutput=True)
    zscr = nc.dram_tensor("zscr", [NB * P * C, HID], F32)

    reps = int(os.environ.get("GCN_REPS", "1"))
    with tile.TileContext(nc) as tc:
        with (
            tc.tile_pool(name="sb", bufs=1) as sb,
            tc.tile_pool(name="stage", bufs=int(os.environ.get("GCN_SBUFS", "3"))) as stage,
            tc.tile_pool(name="idxp", bufs=3) as idxp,
            tc.tile_pool(name="ohp", bufs=3) as ohp,
            tc.tile_pool(name="ps", bufs=1, space="PSUM") as psp,
            tc.tile_pool(name="ps2", bufs=1, space="PSUM") as psp2,
        ):
            acc = sb.tile([P, C, HID], F32, tag="acc")
            z = sb.tile([P, C, HID], F32, tag="z")
            sup_state = {"s0": -1, "tile": None}

            def get_idx(off16, w):
                if sup_state["s0"] < 0 or off16 + w > sup_state["s0"] + SUP:
                    w2 = min(SUP, n_w16 - off16)
                    t = idxp.tile([P, SUP], I16, tag="idx")
                    nc.scalar.dma_start(out=t[:, :w2], in_=idxs[:, off16:off16 + w2])
                    sup_state["s0"] = off16
                    sup_state["tile"] = t
                o = off16 - sup_state["s0"]
                return sup_state["tile"][:, o:o + w]

            gq = [0]

            def gather(dst_ap, src_ap, off16, nidx):
                it = get_idx(off16, nidx // 16)
                gi = nc.gpsimd.dma_gather(dst_ap, src_ap, it, nidx, nidx, HID,
                                          queue_num=gq[0] % NQ)
                gq[0] += 1
                return gi

            def body():
              # (indented 2: repeated GCN_REPS times for benchmarking)
              sup_state["s0"] = -1
              dump_insts = []
              for b in range(NB):
                  nc.gpsimd.memset(acc[:], 0.0)
                  for (off16, nidx, pieces) in bank_chunks[b]:
                      st = stage.tile([P, CH // P, HID], F32, tag="st")
                      gather(st[:, : nidx // P, :], ytab[b * VB:(b + 1) * VB, :],
                             off16, nidx)
                      for (gcol, zcol, ncols) in pieces:
                          nc.vector.tensor_tensor(
                              out=acc[:, zcol:zcol + ncols, :],
                              in0=acc[:, zcol:zcol + ncols, :],
                              in1=st[:, gcol:gcol + ncols, :],
                              op=mybir.AluOpType.add)
                  di = nc.scalar.dma_start(
                      out=zscr[b * P * C:(b + 1) * P * C, :],
                      in_=acc[:].rearrange("p c h -> p (c h)"))
                  dump_insts.append(di)
              # merge partials (node order): z = sum_b permute(acc_b)
              nc.gpsimd.memset(z[:], 0.0)
              for b in range(NB):
                  for (off16, nidx, gcol0, zcol0) in merge_chunks[b]:
                      st = stage.tile([P, CH // P, HID], F32, tag="st")
                      gi = gather(st[:, : nidx // P, :],
                                  zscr[b * P * C:(b + 1) * P * C, :], off16, nidx)
                      tile.add_dep_helper(gi.ins, dump_insts[b].ins, sync=True,
                                          reason="merge gather reads zscr dump")
                      nc.vector.tensor_tensor(
                          out=z[:, zcol0:zcol0 + nidx // P, :],
                          in0=z[:, zcol0:zcol0 + nidx // P, :],
                          in1=st[:, : nidx // P, :],
                          op=mybir.AluOpType.add)
              # dinv
              deg_t = sb.tile([P, C], F32)
              nc.scalar.dma_start(out=deg_t[:], in_=degz[:, :])
              dinv = sb.tile([P, C], F32)
              nc.scalar.activation(dinv[:], deg_t[:], mybir.ActivationFunctionType.Sqrt)
              nc.vector.reciprocal(dinv[:], dinv[:])
              brep_t = sb.tile([P, HID], F32)
              nc.scalar.dma_start(out=brep_t[:], in_=brep[:, :])
              for c in range(C):
                  nc.vector.tensor_tensor(
                      out=z[:, c, :], in0=z[:, c, :],
                      in1=dinv[:, c:c + 1].broadcast_to([P, HID]),
                      op=mybir.AluOpType.mult)
                  nc.vector.tensor_tensor(
                      out=z[:, c, :], in0=z[:, c, :], in1=brep_t[:],
                      op=mybir.AluOpType.add)
              zf = z[:].rearrange("p c h -> p (c h)")
              nc.scalar.activation(zf, zf, mybir.ActivationFunctionType.Relu)
              if dbg:
                  nc.scalar.dma_start(out=zdbg[:, :], in_=zf)
              # pooling: one-hot PSUM matmuls
              gl_t = sb.tile([P, C], F32)
              nc.scalar.dma_start(out=gl_t[:], in_=gl[:, :])
              iota_t = sb.tile([P, HID], F32)
              nc.scalar.dma_start(out=iota_t[:], in_=iota[:, :])
              ones_t = sb.tile([P, 1], F32)
              nc.gpsimd.memset(ones_t[:], 1.0)
              ps_sum = psp.tile([HID, HID], F32, space="PSUM", tag="pssum")
              ps_cnt = psp.tile([HID, 1], F32, space="PSUM", tag="pscnt")
              for c in range(C):
                  oh = ohp.tile([P, HID], F32, tag="oh")
                  nc.vector.tensor_tensor(
                      out=oh[:], in0=gl_t[:, c:c + 1].broadcast_to([P, HID]),
                      in1=iota_t[:], op=mybir.AluOpType.is_equal)
                  nc.tensor.matmul(out=ps_sum[:], lhsT=oh[:], rhs=z[:, c, :],
                                   start=(c == 0), stop=(c == C - 1),
                                   skip_group_check=True)
                  nc.tensor.matmul(out=ps_cnt[:], lhsT=oh[:], rhs=ones_t[:],
                                   start=(c == 0), stop=(c == C - 1),
                                   skip_group_check=True)
              cnt = sb.tile([HID, 1], F32)
              nc.vector.tensor_scalar_max(cnt[:], ps_cnt[:], 1.0)
              nc.vector.reciprocal(cnt[:], cnt[:])
              g_sb = sb.tile([HID, HID], F32)
              nc.vector.tensor_tensor(out=g_sb[:], in0=ps_sum[:],
                                      in1=cnt[:].broadcast_to([HID, HID]),
                                      op=mybir.AluOpType.mult)
              # MLP with homogeneous-coordinate bias
              iden_t = sb.tile([P, P], F32)
              nc.scalar.dma_start(out=iden_t[:], in_=iden[:, :])
              w1_t = sb.tile([P, HID], F32)
              nc.scalar.dma_start(out=w1_t[:], in_=w1a[:, :])
              w2_t = sb.tile([P, 2], F32)
              nc.scalar.dma_start(out=w2_t[:], in_=w2a[:, :])
              gT = psp2.tile([HID, HID], F32, space="PSUM", tag="tr")
              nc.tensor.transpose(out=gT[:], in_=g_sb[:], identity=iden_t[:HID, :HID])
              a1 = sb.tile([P, HID], F32)
              nc.gpsimd.memset(a1[HID:HID + 1, :], 1.0)
              nc.vector.tensor_copy(a1[:HID, :], gT[:])
              h_ps = psp2.tile([HID, HID], F32, space="PSUM", tag="mm")
              nc.tensor.matmul(out=h_ps[:], lhsT=a1[0:HID + 1, :], rhs=w1_t[0:HID + 1, :],
                               start=True, stop=True)
              h_sb = sb.tile([HID, HID], F32)
              nc.scalar.activation(h_sb[:], h_ps[:], mybir.ActivationFunctionType.Relu)
              hT = psp2.tile([HID, HID], F32, space="PSUM", tag="tr2")
              nc.tensor.transpose(out=hT[:], in_=h_sb[:], identity=iden_t[:HID, :HID])
              a2 = sb.tile([P, HID], F32)
              nc.gpsimd.memset(a2[HID:HID + 1, :], 1.0)
              nc.vector.tensor_copy(a2[:HID, :], hT[:])
              o_ps = psp2.tile([HID, 2], F32, space="PSUM", tag="mm2")
              nc.tensor.matmul(out=o_ps[:], lhsT=a2[0:HID + 1, :], rhs=w2_t[0:HID + 1, :],
                               start=True, stop=True)
              o_sb = sb.tile([HID, 2], F32)
              nc.scalar.activation(o_sb[:], o_ps[:], mybir.ActivationFunctionType.Sigmoid)
              nc.scalar.dma_start(out=out[:, :], in_=o_sb[:])

            for _rep in range(reps):
                body()
    return nc


# ---------------------------------------------------------------- jit chain


def _build_chain(nc2, BLK, reps_mm=1):
    """Two jitted steps over the 8-core mesh (the neuronx hook requires the
    bass_exec custom call to be alone in its XLA module):
      A: x_shard [BLK,128]bf16 --matmul--> y [BLK,64]f32 --all_gather-->
         ytab [8*BLK,64] (replicated device array)
      B: bass launch2 (ytab replicated in_spec, everything else core-sharded)
    Data stays device-resident between A and B."""
    import jax
    import jax.numpy as jnp
    from concourse import bass2jax as b2j
    from jax.sharding import NamedSharding as _NS

    b2j.install_neuronx_cc_hook()
    partition_name = (nc2.partition_id_tensor.name
                      if nc2.partition_id_tensor else None)
    in_names, out_names, out_avals = [], [], []
    for alloc in nc2.m.functions[0].allocations:
        if not isinstance(alloc, mybir.MemoryLocationSet):
            continue
        name = alloc.memorylocations[0].name
        if alloc.kind == "ExternalInput":
            if name != partition_name:
                in_names.append(name)
        elif alloc.kind == "ExternalOutput":
            shape = tuple(alloc.tensor_shape)
            dtype = mybir.dt.np(alloc.dtype)
            out_names.append(name)
            out_avals.append(jax.core.ShapedArray(shape, dtype))
    assert in_names[0] == "ytab"
    all_in = list(in_names) + out_names + ([partition_name] if partition_name else [])

    devices = jax.devices()[:NC]
    mesh = b2j.Mesh(np.asarray(devices), ("core",))
    PSpec = b2j.PartitionSpec

    def _mmag(x_sh, W):
        y = x_sh
        for _ in range(reps_mm):
            y = jnp.dot(x_sh, W, preferred_element_type=jnp.float32)
        return jax.lax.all_gather(y, "core", tiled=True)     # [8*BLK, 64]

    jit_mmag = jax.jit(
        b2j.shard_map(_mmag, mesh=mesh, in_specs=(PSpec("core"), PSpec(None)),
                      out_specs=PSpec(None), check_rep=False))

    def _l2(*ops):
        operands = list(ops)
        if partition_name is not None:
            operands.append(b2j.partition_id_tensor())
        outs = b2j._bass_exec_p.bind(
            *operands, out_avals=tuple(out_avals), in_names=tuple(all_in),
            out_names=tuple(out_names), lowering_input_output_aliases=(),
            sim_require_finite=True, sim_require_nnan=True, nc=nc2)
        return tuple(outs)

    n_other = len(in_names) - 1
    jit_l2 = jax.jit(
        b2j.shard_map(
            _l2, mesh=mesh,
            in_specs=(PSpec(None),) + tuple(PSpec("core") for _ in range(
                n_other + len(out_avals))),
            out_specs=tuple(PSpec("core") for _ in out_avals),
            check_rep=False),
        donate_argnums=tuple(range(1 + n_other, 1 + n_other + len(out_avals))),
        keep_unused=True)

    other_names = in_names[1:]

    def prepare(x_list, W_np, in2_list):
        xs = jax.device_put(np.concatenate(x_list, axis=0),
                            _NS(mesh, PSpec("core")))
        Wd = jax.device_put(W_np, _NS(mesh, PSpec(None)))
        l2 = [
            jax.device_put(
                np.concatenate([np.asarray(m[n]) for m in in2_list], axis=0),
                _NS(mesh, PSpec("core")))
            for n in other_names
        ]
        for a in [xs, Wd] + l2:
            a.block_until_ready()

        def run():
            ytab = jit_mmag(xs, Wd)
            zs = [np.zeros((NC * a.shape[0], *a.shape[1:]), a.dtype)
                  for a in out_avals]
            outs = jit_l2(ytab, *l2, *zs)
            for o in outs:
                o.block_until_ready()
            return dict(zip(out_names, outs))

        return run

    return prepare


# ================================================================ v2b: ap_gather quad


NB2 = 8          # banks = cores (one per Q7 core / 16-partition group)
NQT = 4          # target quarters (16 whole graphs each)
CH3 = 1024       # gather chunk slots per ap_gather


def _build_launch3(BLK, NLQ, GL, q_chunks, m_off, n_w16):
    """ap_gather aggregation, quad-packed bf16 feature layout.
    q_chunks: [q] -> list of (slot_off, nidx, [(gcol, acol, ncols)..])
    m_off:    [q] -> slot_off of the NLQ-long merge stream of quarter q
    n_w16: idx tensor columns (= ceil(n_slots_total/16))."""
    nc = bacc.Bacc("TRN2", target_bir_lowering=False, debug=False)
    NLOC2 = NQT * NLQ
    GPQ = NLQ // GL                      # graphs per quarter (16)
    MS = 4 * GL                          # merge sub-chunk slots (4 graphs)
    MP = 8                               # matmul pieces per sub-chunk
    PW = MS // MP                        # slots per matmul piece
    assert MS % MP == 0 and PW * 4 <= 512
    tabd = nc.declare_dram_parameter("tabd", [P, BLK * 4], mybir.dt.bfloat16,
                                     isOutput=False)
    selftabd = nc.declare_dram_parameter("selftabd", [16, NLOC2 * 4],
                                         mybir.dt.bfloat16, isOutput=False)
    idxs = nc.declare_dram_parameter("idxs", [P, n_w16], I16, isOutput=False)
    dinvq4d = nc.declare_dram_parameter("dinvq4d", [1, NLOC2 * 4],
                                        mybir.dt.bfloat16, isOutput=False)
    ones16d = nc.declare_dram_parameter("ones16d", [P, 16], mybir.dt.bfloat16,
                                        isOutput=False)
    invc4d = nc.declare_dram_parameter("invc4d", [1, GPQ * NQT * 4], F32,
                                       isOutput=False)
    w1a = nc.declare_dram_parameter("w1a", [P, HID], F32, isOutput=False)
    w2a = nc.declare_dram_parameter("w2a", [P, 2], F32, isOutput=False)
    iden = nc.declare_dram_parameter("iden", [P, P], F32, isOutput=False)
    out = nc.declare_dram_parameter("out", [HID, 2], F32, isOutput=True)
    dbg = os.environ.get("GCN_DEBUG") == "1"
    if dbg:
        zdbg = nc.declare_dram_parameter("zdbg", [16, NLOC2 * 4], F32,
                                         isOutput=True)

    BF = mybir.dt.bfloat16
    reps = int(os.environ.get("GCN_REPS", "1"))
    with tile.TileContext(nc) as tc:
        with (
            tc.tile_pool(name="sb", bufs=1) as sb,
            tc.tile_pool(name="gb", bufs=2) as gbp,
            tc.tile_pool(name="idxp", bufs=3) as idxp,
            tc.tile_pool(name="msp", bufs=2) as msp,
            tc.tile_pool(name="sc", bufs=2) as scp,
            tc.tile_pool(name="zc", bufs=2) as zcp,
            tc.tile_pool(name="dv", bufs=2) as dvp,
            tc.tile_pool(name="ps", bufs=4, space="PSUM") as psp,
            tc.tile_pool(name="ps2", bufs=2, space="PSUM") as psp2,
        ):
            tab = sb.tile([P, BLK, 4], BF, tag="tab")
            nc.scalar.dma_start(out=tab[:].rearrange("p a b -> p (a b)"),
                                in_=tabd[:, :])
            ones16 = sb.tile([P, 16], BF, tag="o16")
            nc.scalar.dma_start(out=ones16[:], in_=ones16d[:, :])
            acc = sb.tile([P, NLQ, 4], BF, tag="acc")
            pooled = sb.tile([16, GPQ * NQT, 4], F32, tag="pooled")
            sup_state = {"s0": -1, "tile": None}

            def get_idx(off16, w):
                if sup_state["s0"] < 0 or off16 + w > sup_state["s0"] + SUP:
                    w2 = min(SUP, n_w16 - off16)
                    t = idxp.tile([P, SUP], I16, tag="idx")
                    nc.scalar.dma_start(out=t[:, :w2], in_=idxs[:, off16:off16 + w2])
                    sup_state["s0"] = off16
                    sup_state["tile"] = t
                o = off16 - sup_state["s0"]
                return sup_state["tile"][:, o:o + w]

            def body():
              sup_state["s0"] = -1
              for q in range(NQT):
                nc.vector.memset(acc[:], 0.0)
                for (soff, ln, pieces) in q_chunks[q]:
                    it = get_idx(soff // 16, ln // 16)
                    g = gbp.tile([P, CH3, 4], BF, tag="g")
                    nc.gpsimd.ap_gather(g[:, :ln, :], tab[:], it,
                                        channels=P, num_elems=BLK, d=4,
                                        num_idxs=ln)
                    for (gcol, acol, ncols) in pieces:
                        nc.vector.tensor_tensor(
                            out=acc[:, acol:acol + ncols, :],
                            in0=acc[:, acol:acol + ncols, :],
                            in1=g[:, gcol:gcol + ncols, :],
                            op=mybir.AluOpType.add)
                # merge per sub-chunk of MS slots (4 whole graphs)
                for sc in range(NLQ // MS):
                    s0 = sc * MS                       # slot offset in quarter
                    it = get_idx((m_off[q] + s0) // 16, MS // 16)
                    mg = msp.tile([P, MS, 4], BF, tag="mg")
                    nc.gpsimd.ap_gather(mg[:], acc[:], it,
                                        channels=P, num_elems=NLQ, d=4,
                                        num_idxs=MS)
                    self_t = scp.tile([16, MS, 4], BF, tag="self")
                    g0 = (q * NLQ + s0) * 4
                    nc.scalar.dma_start(
                        out=self_t[:].rearrange("p a b -> p (a b)"),
                        in_=selftabd[:, g0:g0 + MS * 4])
                    dv_t = dvp.tile([1, MS * 4], BF, tag="dv")
                    nc.scalar.dma_start(out=dv_t[:], in_=dinvq4d[:, g0:g0 + MS * 4])
                    zch = zcp.tile([16, MS, 4], BF, tag="zch")
                    for p_i in range(MP):
                        ps_t = psp.tile([16, PW * 4], F32, space="PSUM", tag="mps")
                        nc.tensor.matmul(
                            out=ps_t[:], lhsT=ones16[:],
                            rhs=mg[:, p_i * PW:(p_i + 1) * PW, :].rearrange(
                                "p a b -> p (a b)"),
                            start=True, stop=True)
                        nc.vector.tensor_tensor(
                            out=zch[:, p_i * PW:(p_i + 1) * PW, :].rearrange(
                                "p a b -> p (a b)"),
                            in0=ps_t[:],
                            in1=dv_t[:, p_i * PW * 4:(p_i + 1) * PW * 4
                                     ].broadcast_to([16, PW * 4]),
                            op=mybir.AluOpType.mult)
                    nc.vector.tensor_tensor(out=zch[:], in0=zch[:], in1=self_t[:],
                                            op=mybir.AluOpType.add)
                    zfl = zch[:].rearrange("p a b -> p (a b)")
                    nc.scalar.activation(zfl, zfl,
                                         mybir.ActivationFunctionType.Relu)
                    if dbg:
                        zf32 = scp.tile([16, MS * 4], F32, tag="zf32")
                        nc.vector.tensor_copy(zf32[:], zfl)
                        nc.scalar.dma_start(out=zdbg[:, g0:g0 + MS * 4],
                                            in_=zf32[:])
                    for gi in range(MS // GL):
                        gg = q * GPQ + (s0 // GL) + gi
                        nc.vector.tensor_reduce(
                            out=pooled[:, gg, :],
                            in_=zch[:, gi * GL:(gi + 1) * GL, :].rearrange(
                                "p a b -> p b a"),
                            axis=mybir.AxisListType.X,
                            op=mybir.AluOpType.add)
              # mean + MLP tail
              invc = sb.tile([1, GPQ * NQT * 4], F32)
              nc.scalar.dma_start(out=invc[:], in_=invc4d[:, :])
              pfl = pooled[:].rearrange("p a b -> p (a b)")
              nc.vector.tensor_tensor(out=pfl, in0=pfl,
                                      in1=invc[:].broadcast_to([16, GPQ * NQT * 4]),
                                      op=mybir.AluOpType.mult)
              iden_t = sb.tile([P, P], F32)
              nc.scalar.dma_start(out=iden_t[:], in_=iden[:, :])
              g_sb = sb.tile([HID, HID], F32)
              for j in range(4):
                  tps = psp2.tile([HID, 16], F32, space="PSUM", tag="tp")
                  nc.tensor.transpose(out=tps[:], in_=pooled[:, :, j],
                                      identity=iden_t[:16, :16])
                  nc.vector.tensor_copy(g_sb[:, j * 16:(j + 1) * 16], tps[:])
              w1_t = sb.tile([P, HID], F32)
              nc.scalar.dma_start(out=w1_t[:], in_=w1a[:, :])
              w2_t = sb.tile([P, 2], F32)
              nc.scalar.dma_start(out=w2_t[:], in_=w2a[:, :])
              gT = psp2.tile([HID, HID], F32, space="PSUM", tag="tr")
              nc.tensor.transpose(out=gT[:], in_=g_sb[:], identity=iden_t[:HID, :HID])
              a1 = sb.tile([P, HID], F32)
              nc.gpsimd.memset(a1[HID:HID + 1, :], 1.0)
              nc.vector.tensor_copy(a1[:HID, :], gT[:])
              h_ps = psp2.tile([HID, HID], F32, space="PSUM", tag="mm")
              nc.tensor.matmul(out=h_ps[:], lhsT=a1[0:HID + 1, :],
                               rhs=w1_t[0:HID + 1, :], start=True, stop=True)
              h_sb = sb.tile([HID, HID], F32)
              nc.scalar.activation(h_sb[:], h_ps[:],
                                   mybir.ActivationFunctionType.Relu)
              hT = psp2.tile([HID, HID], F32, space="PSUM", tag="tr2")
              nc.tensor.transpose(out=hT[:], in_=h_sb[:], identity=iden_t[:HID, :HID])
              a2 = sb.tile([P, HID], F32)
              nc.gpsimd.memset(a2[HID:HID + 1, :], 1.0)
              nc.vector.tensor_copy(a2[:HID, :], hT[:])
              o_ps = psp2.tile([HID, 2], F32, space="PSUM", tag="mm2")
              nc.tensor.matmul(out=o_ps[:], lhsT=a2[0:HID + 1, :],
                               rhs=w2_t[0:HID + 1, :], start=True, stop=True)
              o_sb = sb.tile([HID, 2], F32)
              nc.scalar.activation(o_sb[:], o_ps[:],
                                   mybir.ActivationFunctionType.Sigmoid)
              nc.scalar.dma_start(out=out[:, :], in_=o_sb[:])

            for _rep in range(reps):
                body()
    return nc


def _schedules3(x, ei, batch_np, W_gcn, b_gcn, W1, b1, W2, b2):
    """Host preprocessing for the ap_gather (v2b) kernel."""
    N = x.shape[0]
    row = ei[0].astype(np.int64)
    col = ei[1].astype(np.int64)
    deg = np.bincount(np.concatenate([col, np.arange(N)]),
                      minlength=N).astype(np.float32)
    dinv = (1.0 / np.sqrt(deg)).astype(np.float32)

    gpc = G // NC
    gb = np.searchsorted(batch_np, np.arange(0, G + 1, gpc))
    Ncs = np.diff(gb)
    C = int((Ncs.max() + P - 1) // P)
    BLK = (C + 1) * P

    gsz = np.bincount(batch_np, minlength=G)
    GL = int(gsz.max())
    NLQ = NQT * 4 * GL // 4 * 1
    NLQ = (G // NC // NQT) * GL          # 16 graphs * GL
    NLOC2 = NQT * NLQ
    assert BLK <= 32768 and NLQ <= 32768

    # node -> (core, slot); slot space per core
    slot = np.empty(N, np.int64)
    starts = np.searchsorted(batch_np, np.arange(G + 1))
    for g in range(G):
        j = g % gpc
        lo, hi = starts[g], starts[g + 1]
        slot[lo:hi] = j * GL + np.arange(hi - lo)

    src_core = np.searchsorted(gb, row, side="right") - 1
    src_loc = row - gb[src_core]               # compact row in owner block
    tgt_core = np.searchsorted(gb, col, side="right") - 1
    tgt_slot = slot[col]

    core_edges = []
    for c in range(NC):
        m = tgt_core == c
        core_edges.append((src_core[m], src_loc[m], tgt_slot[m]))

    # common round sizes per quarter (max over core x bank)
    cnts = np.zeros((NQT, NC, NB2, NLQ), np.int32)
    for c in range(NC):
        sb_, ss_, ts_ = core_edges[c]
        q = ts_ // NLQ
        sq = ts_ % NLQ
        np.add.at(cnts, (q, np.full_like(q, c), sb_, sq), 1)
    nbr_q = []
    for q in range(NQT):
        Rr = int(cnts[q].max())
        nr = np.zeros(Rr, np.int64)
        for r in range(Rr):
            nr[r] = int((cnts[q] > r).sum(axis=2).max())
        nr = ((nr + 3) // 4) * 4
        nbr_q.append(nr)

    q_chunks, q_off = [], []
    off = 0
    for q in range(NQT):
        nr = nbr_q[q]
        S = int(nr.sum())
        S = ((S + 15) // 16) * 16          # keep 16-alignment for idx wrap
        st = np.concatenate([[0], np.cumsum(nr)])
        chunks = []
        pos = 0
        while pos < int(nr.sum()):
            ln = min(CH3, int(nr.sum()) - pos)
            ln = ((ln + 15) // 16) * 16 if ln % 16 else ln
            ln = min(ln, int(nr.sum()) - pos)
            if ln % 16:                     # pad tail chunk to x16
                ln_pad = ((ln + 15) // 16) * 16
            else:
                ln_pad = ln
            pieces = []
            for r in range(len(nr)):
                a = max(pos, st[r]); e = min(pos + ln, st[r + 1])
                if a < e:
                    pieces.append((int(a - pos), int(a - st[r]), int(e - a)))
            chunks.append((off + pos, int(ln_pad), pieces))
            pos += ln
        q_chunks.append(chunks)
        q_off.append(off)
        off += S
    m_off = []
    for q in range(NQT):
        m_off.append(off)
        off += NLQ
    n_slots_total = off
    n_w16 = (n_slots_total + 15) // 16

    # per-core streams
    idx_list, dinvq4_list, selfperm = [], [], []
    for c in range(NC):
        sb_, ss_, ts_ = core_edges[c]
        stream = np.full((NB2, n_w16 * 16), BLK - 1, np.int16)
        for q in range(NQT):
            nr = nbr_q[q]
            st = np.concatenate([[0], np.cumsum(nr)])
            mq = (ts_ // NLQ) == q
            for b in range(NB2):
                m = mq & (sb_ == b)
                sq = ts_[m] % NLQ
                srcs = ss_[m]
                dgb = np.bincount(sq, minlength=NLQ)
                order = np.argsort(-dgb, kind="stable")
                rank = np.empty(NLQ, np.int64)
                rank[order] = np.arange(NLQ)
                rk = rank[sq]
                o = np.lexsort((np.arange(len(rk)), rk))
                rk_s, src_s = rk[o], srcs[o]
                grp = np.searchsorted(rk_s, rk_s)
                j = np.arange(len(rk_s)) - grp
                stream[b, q_off[q] + st[j] + rk_s] = src_s.astype(np.int16)
                stream[b, m_off[q]: m_off[q] + NLQ] = rank.astype(np.int16)
        # wrap16 per group -> [128, n_w16]
        idxw = np.empty((P, n_w16), np.int16)
        for b in range(NB2):
            idxw[16 * b:16 * (b + 1)] = stream[b].reshape(n_w16, 16).T
        idx_list.append(idxw)
        # slot-space dinv (quad-duplicated) + slot->compact map
        lo, hi = int(gb[c]), int(gb[c + 1])
        ns = np.full(NLOC2, -1, np.int64)
        for g in range(c * gpc, (c + 1) * gpc):
            j = g % gpc
            lo_g, hi_g = starts[g], starts[g + 1]
            ns[j * GL: j * GL + (hi_g - lo_g)] = np.arange(lo_g, hi_g)
        isr = ns >= 0
        dslot = np.where(isr, dinv[np.maximum(ns, 0)], 0.0).astype(np.float32)
        dinvq4_list.append(np.repeat(dslot, 4)[None, :].astype(BF16))
        perm = np.where(isr, ns - lo, BLK - 1).astype(np.int32)
        selfperm.append((perm, (dslot * dinv[np.clip(ns, 0, None)]
                                if False else
                                np.where(isr, dinv[np.maximum(ns, 0)] ** 2, 0.0)
                                ).astype(np.float32),
                         isr.astype(np.float32)))

    # x staged compact (dinv_src-scaled), padded to BLK
    x_list = []
    for c in range(NC):
        lo, hi = int(gb[c]), int(gb[c + 1])
        xs = np.zeros((BLK, x.shape[1]), np.float32)
        xs[: hi - lo] = x[lo:hi] * dinv[lo:hi, None]
        x_list.append(xs.astype(BF16))

    # small tensors
    GPQ = NLQ // GL
    ones16 = np.zeros((P, 16), np.float32)
    ones16[np.arange(P), np.arange(P) % 16] = 1.0
    invc4 = np.zeros((NC, 1, GPQ * NQT * 4), np.float32)
    for c in range(NC):
        cg = gsz[c * gpc:(c + 1) * gpc].astype(np.float32)
        invc4[c, 0] = np.repeat(1.0 / np.maximum(cg, 1.0), 4)
    # W1 rows permuted for g' column order (col j*16+q holds feature 4q+j)
    w1p = np.zeros((P, HID), np.float32)
    for jj in range(4):
        for qq in range(16):
            w1p[jj * 16 + qq] = W1[4 * qq + jj]
    w1a = np.zeros((P, HID), np.float32); w1a[:HID] = w1p[:HID]; w1a[HID] = b1
    w2a = np.zeros((P, 2), np.float32); w2a[:HID] = W2; w2a[HID] = b2
    iden = np.eye(P, dtype=np.float32)

    in3 = []
    for c in range(NC):
        in3.append({
            "idxs": idx_list[c],
            "dinvq4d": dinvq4_list[c],
            "ones16d": ones16.astype(BF16),
            "invc4d": invc4[c],
            "w1a": w1a, "w2a": w2a, "iden": iden,
        })
    return dict(C=C, BLK=BLK, GL=GL, NLQ=NLQ, NLOC2=NLOC2, gb=gb,
                q_chunks=q_chunks, m_off=m_off, n_w16=n_w16,
                x_list=x_list, in3=in3, selfperm=selfperm,
                b_gcn=b_gcn.astype(np.float32),
                n_slots_total=n_slots_total)


def _build_chain3(nc3, BLK, NLOC2, selfperm, b_gcn, reps_mm=1):
    """jit A: matmul + quad-pack + all_gather + selftab; jit B: launch3."""
    import jax
    import jax.numpy as jnp
    from concourse import bass2jax as b2j
    from jax.sharding import NamedSharding as _NS

    b2j.install_neuronx_cc_hook()
    partition_name = (nc3.partition_id_tensor.name
                      if nc3.partition_id_tensor else None)
    in_names, out_names, out_avals = [], [], []
    for alloc in nc3.m.functions[0].allocations:
        if not isinstance(alloc, mybir.MemoryLocationSet):
            continue
        name = alloc.memorylocations[0].name
        if alloc.kind == "ExternalInput":
            if name != partition_name:
                in_names.append(name)
        elif alloc.kind == "ExternalOutput":
            shape = tuple(alloc.tensor_shape)
            dtype = mybir.dt.np(alloc.dtype)
            out_names.append(name)
            out_avals.append(jax.core.ShapedArray(shape, dtype))
    assert in_names[0] == "tabd" and in_names[1] == "selftabd"
    all_in = list(in_names) + out_names + ([partition_name] if partition_name else [])

    devices = jax.devices()[:NC]
    mesh = b2j.Mesh(np.asarray(devices), ("core",))
    PSpec = b2j.PartitionSpec

    bias = jnp.asarray(b_gcn)

    def _mmag(x_sh, W, perm, d2s, isr):
        y = x_sh
        for _ in range(reps_mm):
            y = jnp.dot(x_sh, W, preferred_element_type=jnp.float32)  # [BLK, 64]
        tb = y.reshape(BLK, 16, 4).transpose(1, 0, 2).astype(jnp.bfloat16)
        tab = jax.lax.all_gather(tb, "core", tiled=True)   # [128, BLK, 4]
        yp = jnp.take(y, perm, axis=0)                     # [NLOC2, 64]
        st = yp * d2s[:, None] + bias[None, :] * isr[:, None]
        stq = st.reshape(NLOC2, 16, 4).transpose(1, 0, 2).astype(jnp.bfloat16)
        return tab.reshape(P, BLK * 4), stq.reshape(16, NLOC2 * 4)

    jit_mmag = jax.jit(
        b2j.shard_map(_mmag, mesh=mesh,
                      in_specs=(PSpec("core"), PSpec(None), PSpec("core"),
                                PSpec("core"), PSpec("core")),
                      out_specs=(PSpec(None), PSpec("core")),
                      check_rep=False))

    def _l3(*ops):
        operands = list(ops)
        if partition_name is not None:
            operands.append(b2j.partition_id_tensor())
        outs = b2j._bass_exec_p.bind(
            *operands, out_avals=tuple(out_avals), in_names=tuple(all_in),
            out_names=tuple(out_names), lowering_input_output_aliases=(),
            sim_require_finite=True, sim_require_nnan=True, nc=nc3)
        return tuple(outs)

    n_other = len(in_names) - 2
    jit_l3 = jax.jit(
        b2j.shard_map(
            _l3, mesh=mesh,
            in_specs=(PSpec(None),) + tuple(PSpec("core") for _ in range(
                1 + n_other + len(out_avals))),
            out_specs=tuple(PSpec("core") for _ in out_avals),
            check_rep=False),
        donate_argnums=tuple(range(2 + n_other, 2 + n_other + len(out_avals))),
        keep_unused=True)

    other_names = in_names[2:]

    def prepare(x_list, W_np, in3_list):
        xs = jax.device_put(np.concatenate(x_list, axis=0),
                            _NS(mesh, PSpec("core")))
        Wd = jax.device_put(W_np, _NS(mesh, PSpec(None)))
        permd = jax.device_put(np.concatenate([p for (p, _, _) in selfperm]),
                               _NS(mesh, PSpec("core")))
        d2sd = jax.device_put(np.concatenate([d for (_, d, _) in selfperm]),
                              _NS(mesh, PSpec("core")))
        isrd = jax.device_put(np.concatenate([i for (_, _, i) in selfperm]),
                              _NS(mesh, PSpec("core")))
        l3 = [
            jax.device_put(
                np.concatenate([np.asarray(m[n]) for m in in3_list], axis=0),
                _NS(mesh, PSpec("core")))
            for n in other_names
        ]
        for a in [xs, Wd, permd, d2sd, isrd] + l3:
            a.block_until_ready()

        def run():
            tab, stq = jit_mmag(xs, Wd, permd, d2sd, isrd)
            zs = [np.zeros((NC * a.shape[0], *a.shape[1:]), a.dtype)
                  for a in out_avals]
            outs = jit_l3(tab, stq, *l3, *zs)
            for o in outs:
                o.block_until_ready()
            return dict(zip(out_names, outs))

        return run

    return prepare


# ---------------------------------------------------------------- host glue


def _wrap16(vals):
    """int16 stream -> [128, ceil(n/16)] ucode layout (16-wrapped, 8x repl)."""
    n = len(vals)
    w = (n + 15) // 16
    a = np.full(w * 16, -1, np.int16)
    a[:n] = vals
    blk = a.reshape(w, 16).T
    return np.tile(blk, (8, 1))


def _schedules(x, ei, batch_np, W_gcn, b_gcn, W1, b1, W2, b2):
    """All host preprocessing. Returns everything needed to build + run."""
    N = x.shape[0]
    row = ei[0].astype(np.int64)
    col = ei[1].astype(np.int64)
    sl = np.arange(N, dtype=np.int64)
    row2 = np.concatenate([row, sl])
    col2 = np.concatenate([col, sl])
    deg = np.bincount(col2, minlength=N).astype(np.float32)  # >=1 always
    dinv = 1.0 / np.sqrt(deg)

    gpc = G // NC
    gb = np.searchsorted(batch_np, np.arange(0, G + 1, gpc))
    Ncs = np.diff(gb)
    C = int((Ncs.max() + P - 1) // P)
    BLK = (C + 1) * P            # per-core block rows in ytab (last 128+ are 0)

    # x staged: dinv-scaled, padded per core, bf16
    x_list = []
    for c in range(NC):
        lo, hi = int(gb[c]), int(gb[c + 1])
        xs = np.zeros((BLK, x.shape[1]), np.float32)
        xs[: hi - lo] = x[lo:hi] * dinv[lo:hi, None]
        x_list.append(xs.astype(BF16))

    # global row -> (bank, bank-local idx): bank = src core pair
    src_core = np.searchsorted(gb, row2, side="right") - 1
    bankrow = src_core * BLK + (row2 - gb[src_core])
    bank_of = src_core // 2
    bidx_of = bankrow - bank_of * (2 * BLK)

    core_data = []
    for c in range(NC):
        lo, hi = int(gb[c]), int(gb[c + 1])
        m = (col2 >= lo) & (col2 < hi)
        cl = (col2[m] - lo).astype(np.int64)
        core_data.append((lo, hi, bank_of[m], bidx_of[m], cl))

    # common round schedule per bank: N_br = max over cores of roundup128(n_br)
    nbr_all = []
    for b in range(NB):
        per_core = []
        for c in range(NC):
            lo, hi, bank, bidx, cl = core_data[c]
            nloc = hi - lo
            degb = np.bincount(cl[bank == b], minlength=nloc)
            if degb.max() == 0:
                per_core.append(np.zeros(0, np.int64))
                continue
            h = np.bincount(degb)
            nbr = (nloc - np.cumsum(h))[:len(h) - 1]
            per_core.append(np.asarray(nbr, np.int64))
        nbr_all.append(per_core)
    bank_rounds = []
    for b in range(NB):
        R = max((len(a) for a in nbr_all[b]), default=0)
        Nbr = np.zeros(R, np.int64)
        for a in nbr_all[b]:
            aa = np.zeros(R, np.int64)
            aa[:len(a)] = a
            Nbr = np.maximum(Nbr, ((aa + P - 1) // P) * P)
        bank_rounds.append(Nbr)

    # chunk schedule (common): per bank, chunks of <=CH slots + round pieces
    bank_chunks = []
    bank_off16 = []
    off16 = 0
    for b in range(NB):
        Nbr = bank_rounds[b]
        S = int(Nbr.sum())
        starts = np.concatenate([[0], np.cumsum(Nbr)])
        chunks = []
        pos = 0
        while pos < S:
            ln = min(CH, S - pos)
            pieces = []
            for r in range(len(Nbr)):
                a = max(pos, starts[r]); e = min(pos + ln, starts[r + 1])
                if a < e:
                    pieces.append((int((a - pos) // P), int((a - starts[r]) // P),
                                   int((e - a) // P)))
            chunks.append((off16 + pos // 16, int(ln), pieces))
            pos += ln
        bank_chunks.append(chunks)
        bank_off16.append(off16)
        off16 += S // 16
    # merge chunks (common): C*128 idxs per bank
    merge_chunks = []
    merge_off16 = []
    for b in range(NB):
        Sm = C * P
        chunks = []
        pos = 0
        while pos < Sm:
            ln = min(CH, Sm - pos)
            chunks.append((off16 + pos // 16, int(ln), int(pos // P), int(pos // P)))
            pos += ln
        merge_chunks.append(chunks)
        merge_off16.append(off16)
        off16 += Sm // 16
    n_w16 = off16

    # per-core idx streams + small tensors
    in2 = []
    iota64 = np.tile(np.arange(HID, dtype=np.float32), (P, 1))
    brep = np.tile(b_gcn[None, :], (P, 1)).astype(np.float32)
    w1a = np.zeros((P, HID), np.float32); w1a[:HID] = W1; w1a[HID] = b1
    w2a = np.zeros((P, 2), np.float32); w2a[:HID] = W2; w2a[HID] = b2
    iden = np.eye(P, dtype=np.float32)
    DUM = BLK - 1          # bank-local dummy row (even-core pad block, zeros)
    for c in range(NC):
        lo, hi, bank, bidx, cl = core_data[c]
        nloc = hi - lo
        idxbuf = np.empty(n_w16 * 16, np.int16)
        for b in range(NB):
            Nbr = bank_rounds[b]
            S = int(Nbr.sum())
            starts = np.concatenate([[0], np.cumsum(Nbr)])
            stream = np.full(S, DUM, np.int16)
            mb = bank == b
            rb, clb = bidx[mb], cl[mb]
            degb = np.bincount(clb, minlength=nloc)
            order = np.argsort(-degb, kind="stable")
            rank = np.empty(nloc, np.int64)
            rank[order] = np.arange(nloc)
            rk = rank[clb]
            o = np.lexsort((np.arange(len(rk)), rk))
            rk_s, src_s = rk[o], rb[o]
            grp_start = np.searchsorted(rk_s, rk_s)
            j = np.arange(len(rk_s)) - grp_start
            stream[starts[j] + rk_s] = src_s.astype(np.int16)
            idxbuf[bank_off16[b] * 16: bank_off16[b] * 16 + S] = stream
            # merge idx for this bank: node order -> acc_b row
            jb = rank
            mrow = (jb % P) * C + (jb // P)
            mstream = np.zeros(C * P, np.int16)
            mstream[:nloc] = mrow.astype(np.int16)
            idxbuf[merge_off16[b] * 16: merge_off16[b] * 16 + C * P] = mstream
        idxw = _wrap16(idxbuf)
        dgz = np.ones(C * P, np.float32); dgz[:nloc] = deg[lo:hi]
        glv = np.full(C * P, float(HID), np.float32)
        glv[:nloc] = (batch_np[lo:hi] - c * gpc).astype(np.float32)
        in2.append({
            "idxs": idxw,
            "degz": dgz.reshape(C, P).T.copy(),
            "gl": glv.reshape(C, P).T.copy(),
            "iota": iota64, "brep": brep, "w1a": w1a, "w2a": w2a, "iden": iden,
        })
    return dict(C=C, BLK=BLK, x_list=x_list, in2=in2, gb=gb,
                bank_chunks=bank_chunks, merge_chunks=merge_chunks, n_w16=n_w16)


def kernel(x, edge_index, batch, W_gcn, b_gcn, W1, b1, W2, b2):
    x = np.ascontiguousarray(np.asarray(x, dtype=np.float32))
    ei = np.asarray(edge_index).astype(np.int64)
    batch_np = np.asarray(batch).astype(np.int64)
    W_gcn = np.asarray(W_gcn, np.float32); b_gcn = np.asarray(b_gcn, np.float32)
    W1 = np.asarray(W1, np.float32); b1 = np.asarray(b1, np.float32)
    W2 = np.asarray(W2, np.float32); b2 = np.asarray(b2, np.float32)

    S = _schedules(x, ei, batch_np, W_gcn, b_gcn, W1, b1, W2, b2)
    nc2 = _build_launch2(S["C"], S["BLK"], S["bank_chunks"],
                         S["merge_chunks"], S["n_w16"])
    nc2.compile()
    _split_multiwaits(nc2)
    prepare = _build_chain(nc2, S["BLK"])
    try:
        run = prepare(S["x_list"], W_gcn.astype(BF16), S["in2"])
        outs = run()
    except Exception:
        # transient device-unrecoverable (wedged core); retry once
        import time as _time
        _time.sleep(15)
        run = prepare(S["x_list"], W_gcn.astype(BF16), S["in2"])
        outs = run()
    LAST_RUN_INFO.update(S)
    LAST_RUN_INFO["nc2"] = nc2
    if os.environ.get("GCN_DEBUG") == "1":
        zd = np.asarray(outs["zdbg"])            # [8*128, C*HID]
        LAST_RUN_INFO["zdbg"] = [zd[c * P:(c + 1) * P].reshape(P, S["C"], HID)
                                 for c in range(NC)]
    o = np.asarray(outs["out"])                  # [8*64, 2]
    return o[:G].astype(np.float32)
